# revision 6
# baseline (speedup 1.0000x reference)
"""Chamfer distance kernel for Trainium2 (8 NeuronCores).

Strategy (v5: 1024-wide strips / 4 PSUM slots to kill the 2-slot
latency chain; dense input DMA; memset-sourced PE warmup)
---------------------------------------------------------------------
dist[b,i,j] = ||pred[b,j] - gt[b,i]||.  Mins are taken over *negated
squared* distances (so every reduction is a max); sqrt and the means
happen on the host.

neg_sq[i,j] = 2*gt[i].pred[j] - |gt[i]|^2 - |pred[j]|^2 is produced
directly in PSUM by augmented K=24 bf16 matmuls (fp32 operands split
into bf16 triples; see _build_aug).  Operands are replicated at
partition bases 0/32/64/96; each 1024-col strip issues 2 concurrent
512-col matmuls in two of the four 32-row PE groups (groups alternate
with strip parity so adjacent strips overlap 4-wide in the PE).

v4 used [128,2048] strips = 4 PSUM banks, so only 2 slots fit and the
per-slot serial chain (evict -> MM latency+sems ~0.93us -> evict)
bounded the steady state at ~97us.  v5 uses [128,1024] strips = 2
banks -> 4 slots; the MM round trip hides behind the other 3 slots.
Cost model: ACT evict 1024x0.833+~160 = ~1.01us/strip, DVE fused
evict+rowmax 1024x1.042+~120 = ~1.18us/strip (PSUM fp32 reads are 1x;
bf16 PSUM would enable 2x DVE but is TRN3-only).

Sharding: gt rows split across 8 cores (1024 rows/core/batch = 8 row
tiles of 128).  Per batch, tiles are R-type (5: ACT copy-evict, fp16
span halves ship raw to DRAM, host reduces rows+cols) or F-type (3:
DVE fused evict+rowmax; spans chain-fold via fp16 tensor_tensor 2x and
the fold ships).  ACT 80 strips x 1.01 = 80.8us ~= DVE 48 x 1.18 +
8 folds x 2.29 + accum reads = 78.9us ~= DMA 28.9MB / 0.376MB/us =
76.7us -- three-way balanced.

Startup: input DMA ships only the 96 live partition rows (ag via 4
sync-queue dispatches, apt via 8 gpsimd/SWDGE dispatches split by
(group, batch)), and the PE warmup matmuls read a memset tile instead
of waiting for ag, so real strips start as soon as b0's columns land.
"""

import os
import sys
import numpy as np
import ml_dtypes

# ---------------------------------------------------------------------------
# problem constants (hardcoded per spec: pred/gt [2, 8192, 3] fp32)
B = 2
N = 8192
NCORES = 8
GPC = N // NCORES          # gt rows per core per batch = 1024
RT = GPC // 128            # row tiles per batch per core = 8
SPANW = 4096               # span width (ship/fold unit)
SPB = 2                    # spans per tile (8192 / 4096)
KPS = 4                    # strips per span (4096 / 1024)
W = 1024                   # strip width (2 PSUM banks -> 4 slots)
NSTRIP = B * RT * SPB * KPS  # 128 strips per core
NTILE = B * RT             # 16 row tiles per core
K = 24                     # contraction rows of the augmented matmul

# per-batch tile types: F (DVE fused evict+rowmax, col-side via fold
# chain) at {0,4,6}; R (ACT copy-evict + raw fp16 ship, host reduces)
# elsewhere.  Engine interleave below.
F_T = (0, 4, 6)
R_T = (1, 2, 3, 5, 7)
# Period-16 strip interleave (10 ACT : 6 DVE = 40:24 per 64-strip
# batch, exactly matching 5 R + 3 F tiles x 8 strips).  Chosen by
# discrete-event search over the 4-slot pipeline.
PAT16 = "AADDAADADADADAAA"
# final window of the stream: alternate D/A so the last fold + cmax ship
# happen ~7us before the last eviction and the trailing DMA is covered
PAT16_LAST = "DADADADADADAAAAA"
WARMUP_MM = 4              # PE warmup matmuls (clock is warm by profile
                           # time; these are cheap insurance)

_BF16 = ml_dtypes.bfloat16


def _ensure_concourse():
    for p in ("/root/.axon_site", "/root/.axon_site/_ro/trn_rl_repo",
              "/root/.axon_site/_ro/pypackages", "/opt/trn_rl_repo"):
        if os.path.isdir(p) and p not in sys.path:
            sys.path.append(p)


def _split3(x64):
    """Split a float64 array into three bf16 components summing to ~24 bits."""
    h = x64.astype(_BF16)
    r = x64 - h.astype(np.float64)
    m = r.astype(_BF16)
    r2 = r - m.astype(np.float64)
    l = r2.astype(_BF16)
    return h, m, l


def _build_aug(pred, gt):
    """Build aug_pred [K, B*N] and aug_gt [K, B*N] bf16 host arrays.

    Row pairing k: lhsT[k] (gt side) x rhs[k] (pred side):
      0-2   gh . Ph      3-5   gh . Pm      6-8   gm . Ph
      9-11  gh . Pl     12-14  gl . Ph     15-17  gm . Pm
      18-20 gsq{h,m,l} . (-1)              21-23  1 . (-psq{h,m,l})
    where P = 2*pred.
    """
    g64 = gt.astype(np.float64).reshape(B * N, 3)
    P64 = (2.0 * pred.astype(np.float64)).reshape(B * N, 3)
    gsq = (gt.astype(np.float32) ** 2).sum(-1, dtype=np.float32).astype(np.float64).reshape(B * N)
    psq = (pred.astype(np.float32) ** 2).sum(-1, dtype=np.float32).astype(np.float64).reshape(B * N)

    gh, gm, gl = _split3(g64)
    Ph, Pm, Pl = _split3(P64)
    gsqh, gsqm, gsql = _split3(gsq)
    psqh, psqm, psql = _split3(psq)

    one = np.ones(B * N, _BF16)
    neg1 = np.full(B * N, -1.0, _BF16)

    def rows3(a):  # [B*N, 3] -> 3 rows
        return [a[:, 0], a[:, 1], a[:, 2]]

    aug_gt = np.stack(
        rows3(gh) + rows3(gh) + rows3(gm) + rows3(gh) + rows3(gl) + rows3(gm)
        + [gsqh, gsqm, gsql, one, one, one], axis=0)
    aug_pred = np.stack(
        rows3(Ph) + rows3(Pm) + rows3(Ph) + rows3(Pl) + rows3(Ph) + rows3(Pm)
        + [neg1, neg1, neg1, -psqh, -psqm, -psql], axis=0)
    assert aug_gt.shape == (K, B * N) and aug_pred.shape == (K, B * N)
    return aug_gt, aug_pred


def build_nc():
    """Trace + compile the single-program SPMD kernel. Returns the Bacc."""
    _ensure_concourse()
    from contextlib import ExitStack
    import concourse.tile as tile
    from concourse import bacc, mybir

    f32 = mybir.dt.float32
    bf16 = mybir.dt.bfloat16
    f16 = mybir.dt.float16
    MAX = mybir.AluOpType.max
    ADD = mybir.AluOpType.add

    nc = bacc.Bacc("TRN2", target_bir_lowering=False, debug=False,
                   enable_asserts=False, num_devices=NCORES)
    # aug operands arrive pre-replicated at partition bases 0/32/64/96
    # (dead rows 24-31/56-63/88-95 exist in DRAM but are never DMA'd).
    ag_d = nc.dram_tensor("aug_gt", [96 + K, B * GPC], bf16,
                          kind="ExternalInput").ap()
    ap_d = nc.dram_tensor("aug_pred", [96 + K, B * N], bf16,
                          kind="ExternalInput").ap()
    # F tiles' fused rowmax accums, one column per strip id
    rmax_d = nc.dram_tensor("rowmax_out", [128, NSTRIP], f32, kind="ExternalOutput").ap()
    # chain-folded col-side spans: slot pid = b*2 + sp
    cmax_d = nc.dram_tensor("colmax_out", [128, B * 2 * SPANW], f16,
                            kind="ExternalOutput").ap()
    # raw spans of R tiles: slot = (b*5 + R_T.index(t))*2 + sp
    craw_d = nc.dram_tensor("colraw_out", [128, B * 5 * 2 * SPANW], f16,
                            kind="ExternalOutput").ap()

    with tile.TileContext(nc) as tc, ExitStack() as ctx:
        const_pool = ctx.enter_context(tc.tile_pool(name="const", bufs=1))
        psum_pool = ctx.enter_context(tc.tile_pool(name="ps", bufs=4, space="PSUM"))
        span_pool = ctx.enter_context(tc.tile_pool(name="bs", bufs=2))
        pf_pool = ctx.enter_context(tc.tile_pool(name="pf", bufs=3))

        ag = const_pool.tile([96 + K, B * GPC], bf16)
        apt = const_pool.tile([96 + K, B * N], bf16)
        nc.sync.dma_start(ag[:], ag_d[:])
        for b in range(B):
            for cb in range(4):
                ccol = b * N + cb * 2048
                nc.sync.dma_start(apt[:, ccol:ccol + 2048],
                                  ap_d[:, ccol:ccol + 2048])
        rscr = const_pool.tile([128, NSTRIP], f32)
        nc.vector.memset(rscr[:], -3.0e38)

        # PE HAM warmup: sustained matmul activity (reading only ag, which
        # arrives first) un-throttles the PE clock gate before the real
        # strips start; results are overwritten/ignored.
        pw = psum_pool.tile([128, W], f32, tag="ps")
        for _ in range(WARMUP_MM):
            nc.tensor.matmul(pw[:, :512], lhsT=ag[0:K, 0:128],
                             rhs=ag[0:K, 512:1024], start=True, stop=True,
                             tile_position=(0, 0))

        gstrip = [0]  # global strip counter (selects the PE group pair)

        def emit_strip(b, t, sp, k, span, typ):
            ti = b * RT + t
            s = (ti * SPB + sp) * KPS + k
            ccol = b * N + sp * SPANW + k * W
            wcol = ti * 128
            psum = psum_pool.tile([128, W], f32, tag="ps", name="psum")
            gbase = 64 * (gstrip[0] % 2)
            gstrip[0] += 1
            for j in range(2):
                g = gbase + 32 * j
                nc.tensor.matmul(
                    psum[:, j * 512:(j + 1) * 512],
                    lhsT=ag[g:g + K, wcol:wcol + 128],
                    rhs=apt[g:g + K, ccol + j * 512: ccol + (j + 1) * 512],
                    start=True, stop=True,
                    tile_position=(g, 0))
            out_slice = span[:, k * W:(k + 1) * W]
            if typ == 'F':
                # evict + this strip's rowmax in one 1x DVE pass
                nc.vector.tensor_scalar(
                    out=out_slice, in0=psum[:], scalar1=0.0,
                    scalar2=None, op0=ADD, op1=MAX,
                    accum_out=rscr[:, s:s + 1])
            else:
                nc.scalar.activation(out_slice, psum[:],
                                     mybir.ActivationFunctionType.Copy)

        def ship_half(b, t, sp, h, span):
            # raw R half-span ships on the GPSIMD (SWDGE) DMA queue so
            # they can't head-of-line block the sync queue's fold ships
            slot = ((b * 5 + R_T.index(t)) * 2 + sp) * SPANW
            nc.gpsimd.dma_start(
                craw_d[:, slot + h * 2048: slot + (h + 1) * 2048],
                span[:, h * 2048:(h + 1) * 2048])

        def finish_span_f(b, t, sp, span):
            # chain-fold the F spans of this (b, sp); ship after the third
            key = (b, sp)
            acc, cnt = chain.get(key, (None, 0))
            if acc is None:
                chain[key] = (span, 1)
                return
            pf = pf_pool.tile([128, SPANW], f16, tag="pf")
            nc.vector.tensor_tensor(out=pf[:], in0=acc[:], in1=span[:], op=MAX)
            cnt += 1
            if cnt < len(F_T):
                chain[key] = (pf, cnt)
            else:
                pid = b * 2 + sp
                nc.sync.dma_start(
                    cmax_d[:, pid * SPANW:(pid + 1) * SPANW], pf[:])
                chain.pop(key)

        # Flat per-batch strip stream: DVE-evicted (F) and ACT-evicted
        # (R) strips interleaved 10:6 per 16 so both engines run
        # continuously through the four PSUM slots.
        chain = {}  # (b, sp) -> (accumulated col-side fold tile, count)
        for b in range(B):
            # sp-major so each (b, sp) fold chain completes as early as its
            # last span allows (and early strips touch only sp0's columns)
            astrips = [(t, sp, k) for sp in range(SPB) for t in R_T
                       for k in range(KPS)]
            dstrips = [(t, sp, k) for sp in range(SPB) for t in F_T
                       for k in range(KPS)]
            ai = di = 0
            cur = {}       # (t, sp) -> span being filled
            for i in range(len(astrips) + len(dstrips)):
                last_window = (b == B - 1) and i >= 48
                which = (PAT16_LAST if last_window else PAT16)[i % 16]
                if (which == 'D' and di < len(dstrips)) or ai >= len(astrips):
                    t, sp, k = dstrips[di]
                    di += 1
                    side = 'd'
                else:
                    t, sp, k = astrips[ai]
                    ai += 1
                    side = 'a'
                typ = 'F' if t in F_T else 'R'
                if k == 0:
                    cur[(t, sp)] = span_pool.tile([128, SPANW], f16,
                                                  tag=f"sp_{side}",
                                                  bufs=6 if side == 'a' else 4,
                                                  name="span")
                emit_strip(b, t, sp, k, cur[(t, sp)], typ)
                if typ == 'R' and k in (1, 3):
                    ship_half(b, t, sp, k // 2, cur[(t, sp)])
                    if k == 3:
                        cur.pop((t, sp))
                elif typ == 'F' and k == 3:
                    finish_span_f(b, t, sp, cur.pop((t, sp)))
        nc.sync.dma_start(rmax_d[:], rscr[:])

    nc.compile()
    return nc


_NC_CACHE = None


def _get_nc():
    global _NC_CACHE
    if _NC_CACHE is None:
        _NC_CACHE = build_nc()
    return _NC_CACHE


def _replicate4(a):
    """Replicate [K, cols] to partition bases 0/32/64/96 of a [96+K, cols]."""
    out = np.zeros((96 + K, a.shape[1]), _BF16)
    for g in range(4):
        out[32 * g:32 * g + K] = a
    return out


def make_in_maps(pred, gt):
    """Per-core input dicts. Core c gets gt rows [c*GPC, (c+1)*GPC) of each
    batch (aug_gt columns laid out b-major: (b*RT + t)*128 + p)."""
    aug_gt, aug_pred = _build_aug(pred, gt)
    ag_bn = aug_gt.reshape(K, B, N)
    apr = _replicate4(aug_pred)
    in_maps = []
    for c in range(NCORES):
        ag_c = ag_bn[:, :, c * GPC:(c + 1) * GPC].reshape(K, B * GPC)
        in_maps.append({"aug_gt": _replicate4(ag_c), "aug_pred": apr})
    return in_maps


def finalize(results):
    """Host finale: negated maxes -> mins -> sqrt -> means."""
    dist1_sq = np.empty((B, N), np.float64)
    dist2_parts = []   # per-core [B, 2, SPANW] col-side partial maxes
    for c in range(NCORES):
        rscr = np.asarray(results[c]["rowmax_out"], np.float32)
        # colraw_out layout: [128, ((b*5+ridx)*2+sp)*SPANW + jj]
        craw = np.asarray(results[c]["colraw_out"]).reshape(128, B, 5, 2, SPANW)
        cfold = np.asarray(results[c]["colmax_out"]).reshape(128, B, 2, SPANW)

        rmax = np.empty((B, RT, 128), np.float32)
        for b in range(B):
            for t in F_T:
                s0 = (b * RT + t) * SPB * KPS
                rmax[b, t] = rscr[:, s0:s0 + SPB * KPS].max(axis=1)
            for ridx, t in enumerate(R_T):
                rmax[b, t] = craw[:, b, ridx, :, :].astype(np.float32).max(axis=(1, 2))
        dist1_sq[:, c * GPC:(c + 1) * GPC] = -rmax.reshape(B, GPC).astype(np.float64)

        # col-side: max of chain-fold slots and raw spans over tiles
        raw_part = craw.max(axis=2)                       # [128, B, 2, SPANW]
        part = np.maximum(cfold, raw_part).max(axis=0)    # [B, 2, SPANW]
        dist2_parts.append(part)

    cmax = np.stack(dist2_parts, axis=0).max(axis=0)     # [B, 2, SPANW]
    cmax = cmax.reshape(B, N)                            # sp-major cols
    dist2_sq = -(cmax.astype(np.float64))

    dist1 = np.sqrt(np.maximum(dist1_sq, 0.0))
    dist2 = np.sqrt(np.maximum(dist2_sq, 0.0))
    chamfer = (dist1.mean(axis=1) + dist2.mean(axis=1)).mean()
    return np.float32(chamfer)


def kernel(pred, gt):
    _ensure_concourse()
    pred = np.asarray(pred, dtype=np.float32)
    gt = np.asarray(gt, dtype=np.float32)
    assert pred.shape == (B, N, 3) and gt.shape == (B, N, 3)

    in_maps = make_in_maps(pred, gt)
    nc = _get_nc()
    from concourse import bass_utils
    res = bass_utils.run_bass_kernel_spmd(nc, in_maps, core_ids=list(range(NCORES)))
    return finalize(res.results)


# revision 7
# speedup vs baseline: 1.1004x; 1.1004x over previous
"""Chamfer distance kernel for Trainium2 (8 NeuronCores).

Strategy (v5: 1024-wide strips / 4 PSUM slots to kill the 2-slot
latency chain; dense input DMA; memset-sourced PE warmup)
---------------------------------------------------------------------
dist[b,i,j] = ||pred[b,j] - gt[b,i]||.  Mins are taken over *negated
squared* distances (so every reduction is a max); sqrt and the means
happen on the host.

neg_sq[i,j] = 2*gt[i].pred[j] - |gt[i]|^2 - |pred[j]|^2 is produced
directly in PSUM by augmented K=24 bf16 matmuls (fp32 operands split
into bf16 triples; see _build_aug).  Operands are replicated at
partition bases 0/32/64/96; each 1024-col strip issues 2 concurrent
512-col matmuls in two of the four 32-row PE groups (groups alternate
with strip parity so adjacent strips overlap 4-wide in the PE).

v4 used [128,2048] strips = 4 PSUM banks, so only 2 slots fit and the
per-slot serial chain (evict -> MM latency+sems ~0.93us -> evict)
bounded the steady state at ~97us.  v5 uses [128,1024] strips = 2
banks -> 4 slots; the MM round trip hides behind the other 3 slots.
Cost model: ACT evict 1024x0.833+~160 = ~1.01us/strip, DVE fused
evict+rowmax 1024x1.042+~120 = ~1.18us/strip (PSUM fp32 reads are 1x;
bf16 PSUM would enable 2x DVE but is TRN3-only).

Sharding: gt rows split across 8 cores (1024 rows/core/batch = 8 row
tiles of 128).  Per batch, tiles are R-type (5: ACT copy-evict, fp16
span halves ship raw to DRAM, host reduces rows+cols) or F-type (3:
DVE fused evict+rowmax; spans chain-fold via fp16 tensor_tensor 2x and
the fold ships).  ACT 80 strips x 1.01 = 80.8us ~= DVE 48 x 1.18 +
8 folds x 2.29 + accum reads = 78.9us ~= DMA 28.9MB / 0.376MB/us =
76.7us -- three-way balanced.

Startup: input DMA ships only the 96 live partition rows (ag via 4
sync-queue dispatches, apt via 8 gpsimd/SWDGE dispatches split by
(group, batch)), and the PE warmup matmuls read a memset tile instead
of waiting for ag, so real strips start as soon as b0's columns land.
"""

import os
import sys
import numpy as np
import ml_dtypes

# ---------------------------------------------------------------------------
# problem constants (hardcoded per spec: pred/gt [2, 8192, 3] fp32)
B = 2
N = 8192
NCORES = 8
GPC = N // NCORES          # gt rows per core per batch = 1024
RT = GPC // 128            # row tiles per batch per core = 8
SPANW = 4096               # span width (ship/fold unit)
SPB = 2                    # spans per tile (8192 / 4096)
KPS = 4                    # strips per span (4096 / 1024)
W = 1024                   # strip width (2 PSUM banks -> 4 slots)
NSTRIP = B * RT * SPB * KPS  # 128 strips per core
NTILE = B * RT             # 16 row tiles per core
K = 24                     # contraction rows of the augmented matmul

# per-batch tile types: F (DVE fused evict+rowmax, col-side via fold
# chain) at {0,4,6}; R (ACT copy-evict + raw fp16 ship, host reduces)
# elsewhere.  Engine interleave below.
F_T = (0, 4, 6)
R_T = (1, 2, 3, 5, 7)
# Period-16 strip interleave (10 ACT : 6 DVE = 40:24 per 64-strip
# batch, exactly matching 5 R + 3 F tiles x 8 strips).  Chosen by
# discrete-event search over the 4-slot pipeline.
PAT16 = "AADDAADADADADAAA"
# final window of the stream: alternate D/A so the last fold + cmax ship
# happen ~7us before the last eviction and the trailing DMA is covered
PAT16_LAST = "DADADADADADAAAAA"
# PE HAM warmup matmuls before the main loop.  The burst must be long
# enough (~9us serial) to cross the clock-gate's un-throttle threshold;
# with a short burst the whole core runs ~17% slower for the entire
# execution (measured: MM 724 vs 604, ACT 1336 vs 1139).
WARMUP_MM = 22

_BF16 = ml_dtypes.bfloat16


def _ensure_concourse():
    for p in ("/root/.axon_site", "/root/.axon_site/_ro/trn_rl_repo",
              "/root/.axon_site/_ro/pypackages", "/opt/trn_rl_repo"):
        if os.path.isdir(p) and p not in sys.path:
            sys.path.append(p)


def _split3(x64):
    """Split a float64 array into three bf16 components summing to ~24 bits."""
    h = x64.astype(_BF16)
    r = x64 - h.astype(np.float64)
    m = r.astype(_BF16)
    r2 = r - m.astype(np.float64)
    l = r2.astype(_BF16)
    return h, m, l


def _build_aug(pred, gt):
    """Build aug_pred [K, B*N] and aug_gt [K, B*N] bf16 host arrays.

    Row pairing k: lhsT[k] (gt side) x rhs[k] (pred side):
      0-2   gh . Ph      3-5   gh . Pm      6-8   gm . Ph
      9-11  gh . Pl     12-14  gl . Ph     15-17  gm . Pm
      18-20 gsq{h,m,l} . (-1)              21-23  1 . (-psq{h,m,l})
    where P = 2*pred.
    """
    g64 = gt.astype(np.float64).reshape(B * N, 3)
    P64 = (2.0 * pred.astype(np.float64)).reshape(B * N, 3)
    gsq = (gt.astype(np.float32) ** 2).sum(-1, dtype=np.float32).astype(np.float64).reshape(B * N)
    psq = (pred.astype(np.float32) ** 2).sum(-1, dtype=np.float32).astype(np.float64).reshape(B * N)

    gh, gm, gl = _split3(g64)
    Ph, Pm, Pl = _split3(P64)
    gsqh, gsqm, gsql = _split3(gsq)
    psqh, psqm, psql = _split3(psq)

    one = np.ones(B * N, _BF16)
    neg1 = np.full(B * N, -1.0, _BF16)

    def rows3(a):  # [B*N, 3] -> 3 rows
        return [a[:, 0], a[:, 1], a[:, 2]]

    aug_gt = np.stack(
        rows3(gh) + rows3(gh) + rows3(gm) + rows3(gh) + rows3(gl) + rows3(gm)
        + [gsqh, gsqm, gsql, one, one, one], axis=0)
    aug_pred = np.stack(
        rows3(Ph) + rows3(Pm) + rows3(Ph) + rows3(Pl) + rows3(Ph) + rows3(Pm)
        + [neg1, neg1, neg1, -psqh, -psqm, -psql], axis=0)
    assert aug_gt.shape == (K, B * N) and aug_pred.shape == (K, B * N)
    return aug_gt, aug_pred


def build_nc():
    """Trace + compile the single-program SPMD kernel. Returns the Bacc."""
    _ensure_concourse()
    from contextlib import ExitStack
    import concourse.tile as tile
    from concourse import bacc, mybir

    f32 = mybir.dt.float32
    bf16 = mybir.dt.bfloat16
    f16 = mybir.dt.float16
    MAX = mybir.AluOpType.max
    ADD = mybir.AluOpType.add

    nc = bacc.Bacc("TRN2", target_bir_lowering=False, debug=False,
                   enable_asserts=False, num_devices=NCORES)
    # aug operands arrive pre-replicated at partition bases 0/32/64/96
    # (dead rows 24-31/56-63/88-95 exist in DRAM but are never DMA'd).
    ag_d = nc.dram_tensor("aug_gt", [96 + K, B * GPC], bf16,
                          kind="ExternalInput").ap()
    ap_d = nc.dram_tensor("aug_pred", [96 + K, B * N], bf16,
                          kind="ExternalInput").ap()
    # F tiles' fused rowmax accums, one column per strip id
    rmax_d = nc.dram_tensor("rowmax_out", [128, NSTRIP], f32, kind="ExternalOutput").ap()
    # chain-folded col-side spans: slot pid = b*2 + sp
    cmax_d = nc.dram_tensor("colmax_out", [128, B * 2 * SPANW], f16,
                            kind="ExternalOutput").ap()
    # raw spans of R tiles: slot = (b*5 + R_T.index(t))*2 + sp
    craw_d = nc.dram_tensor("colraw_out", [128, B * 5 * 2 * SPANW], f16,
                            kind="ExternalOutput").ap()

    with tile.TileContext(nc) as tc, ExitStack() as ctx:
        const_pool = ctx.enter_context(tc.tile_pool(name="const", bufs=1))
        psum_pool = ctx.enter_context(tc.tile_pool(name="ps", bufs=4, space="PSUM"))
        span_pool = ctx.enter_context(tc.tile_pool(name="bs", bufs=2))
        pf_pool = ctx.enter_context(tc.tile_pool(name="pf", bufs=3))

        ag = const_pool.tile([96 + K, B * GPC], bf16)
        apt = const_pool.tile([96 + K, B * N], bf16)
        nc.sync.dma_start(ag[:], ag_d[:])
        for b in range(B):
            for cb in range(4):
                ccol = b * N + cb * 2048
                nc.sync.dma_start(apt[:, ccol:ccol + 2048],
                                  ap_d[:, ccol:ccol + 2048])
        rscr = const_pool.tile([128, NSTRIP], f32)
        nc.vector.memset(rscr[:], -3.0e38)

        # PE HAM warmup: sustained matmul activity (reading only ag, which
        # arrives first) un-throttles the PE clock gate before the real
        # strips start; results are overwritten/ignored.
        pw = psum_pool.tile([128, W], f32, tag="ps")
        for _ in range(WARMUP_MM):
            nc.tensor.matmul(pw[:, :512], lhsT=ag[0:K, 0:128],
                             rhs=ag[0:K, 512:1024], start=True, stop=True,
                             tile_position=(0, 0))

        gstrip = [0]  # global strip counter (selects the PE group pair)

        def emit_strip(b, t, sp, k, span, typ):
            ti = b * RT + t
            s = (ti * SPB + sp) * KPS + k
            ccol = b * N + sp * SPANW + k * W
            wcol = ti * 128
            psum = psum_pool.tile([128, W], f32, tag="ps", name="psum")
            gbase = 64 * (gstrip[0] % 2)
            gstrip[0] += 1
            for j in range(2):
                g = gbase + 32 * j
                nc.tensor.matmul(
                    psum[:, j * 512:(j + 1) * 512],
                    lhsT=ag[g:g + K, wcol:wcol + 128],
                    rhs=apt[g:g + K, ccol + j * 512: ccol + (j + 1) * 512],
                    start=True, stop=True,
                    tile_position=(g, 0))
            out_slice = span[:, k * W:(k + 1) * W]
            if typ == 'F':
                # evict + this strip's rowmax in one 1x DVE pass
                nc.vector.tensor_scalar(
                    out=out_slice, in0=psum[:], scalar1=0.0,
                    scalar2=None, op0=ADD, op1=MAX,
                    accum_out=rscr[:, s:s + 1])
            else:
                nc.scalar.activation(out_slice, psum[:],
                                     mybir.ActivationFunctionType.Copy)

        def ship_half(b, t, sp, h, span):
            # raw R half-span ships on the GPSIMD (SWDGE) DMA queue so
            # they can't head-of-line block the sync queue's fold ships
            slot = ((b * 5 + R_T.index(t)) * 2 + sp) * SPANW
            nc.gpsimd.dma_start(
                craw_d[:, slot + h * 2048: slot + (h + 1) * 2048],
                span[:, h * 2048:(h + 1) * 2048])

        def finish_span_f(b, t, sp, span):
            # chain-fold the F spans of this (b, sp); ship after the third
            key = (b, sp)
            acc, cnt = chain.get(key, (None, 0))
            if acc is None:
                chain[key] = (span, 1)
                return
            pf = pf_pool.tile([128, SPANW], f16, tag="pf")
            nc.vector.tensor_tensor(out=pf[:], in0=acc[:], in1=span[:], op=MAX)
            cnt += 1
            if cnt < len(F_T):
                chain[key] = (pf, cnt)
            else:
                pid = b * 2 + sp
                nc.sync.dma_start(
                    cmax_d[:, pid * SPANW:(pid + 1) * SPANW], pf[:])
                chain.pop(key)

        # Flat per-batch strip stream: DVE-evicted (F) and ACT-evicted
        # (R) strips interleaved 10:6 per 16 so both engines run
        # continuously through the four PSUM slots.
        chain = {}  # (b, sp) -> (accumulated col-side fold tile, count)
        for b in range(B):
            # sp-major so each (b, sp) fold chain completes as early as its
            # last span allows (and early strips touch only sp0's columns)
            astrips = [(t, sp, k) for sp in range(SPB) for t in R_T
                       for k in range(KPS)]
            dstrips = [(t, sp, k) for sp in range(SPB) for t in F_T
                       for k in range(KPS)]
            ai = di = 0
            cur = {}       # (t, sp) -> span being filled
            for i in range(len(astrips) + len(dstrips)):
                last_window = (b == B - 1) and i >= 48
                which = (PAT16_LAST if last_window else PAT16)[i % 16]
                if (which == 'D' and di < len(dstrips)) or ai >= len(astrips):
                    t, sp, k = dstrips[di]
                    di += 1
                    side = 'd'
                else:
                    t, sp, k = astrips[ai]
                    ai += 1
                    side = 'a'
                typ = 'F' if t in F_T else 'R'
                if k == 0:
                    cur[(t, sp)] = span_pool.tile([128, SPANW], f16,
                                                  tag=f"sp_{side}",
                                                  bufs=6 if side == 'a' else 4,
                                                  name="span")
                emit_strip(b, t, sp, k, cur[(t, sp)], typ)
                if typ == 'R' and k in (1, 3):
                    ship_half(b, t, sp, k // 2, cur[(t, sp)])
                    if k == 3:
                        cur.pop((t, sp))
                elif typ == 'F' and k == 3:
                    finish_span_f(b, t, sp, cur.pop((t, sp)))
        nc.sync.dma_start(rmax_d[:], rscr[:])

    nc.compile()
    return nc


_NC_CACHE = None


def _get_nc():
    global _NC_CACHE
    if _NC_CACHE is None:
        _NC_CACHE = build_nc()
    return _NC_CACHE


def _replicate4(a):
    """Replicate [K, cols] to partition bases 0/32/64/96 of a [96+K, cols]."""
    out = np.zeros((96 + K, a.shape[1]), _BF16)
    for g in range(4):
        out[32 * g:32 * g + K] = a
    return out


def make_in_maps(pred, gt):
    """Per-core input dicts. Core c gets gt rows [c*GPC, (c+1)*GPC) of each
    batch (aug_gt columns laid out b-major: (b*RT + t)*128 + p)."""
    aug_gt, aug_pred = _build_aug(pred, gt)
    ag_bn = aug_gt.reshape(K, B, N)
    apr = _replicate4(aug_pred)
    in_maps = []
    for c in range(NCORES):
        ag_c = ag_bn[:, :, c * GPC:(c + 1) * GPC].reshape(K, B * GPC)
        in_maps.append({"aug_gt": _replicate4(ag_c), "aug_pred": apr})
    return in_maps


def finalize(results):
    """Host finale: negated maxes -> mins -> sqrt -> means."""
    dist1_sq = np.empty((B, N), np.float64)
    dist2_parts = []   # per-core [B, 2, SPANW] col-side partial maxes
    for c in range(NCORES):
        rscr = np.asarray(results[c]["rowmax_out"], np.float32)
        # colraw_out layout: [128, ((b*5+ridx)*2+sp)*SPANW + jj]
        craw = np.asarray(results[c]["colraw_out"]).reshape(128, B, 5, 2, SPANW)
        cfold = np.asarray(results[c]["colmax_out"]).reshape(128, B, 2, SPANW)

        rmax = np.empty((B, RT, 128), np.float32)
        for b in range(B):
            for t in F_T:
                s0 = (b * RT + t) * SPB * KPS
                rmax[b, t] = rscr[:, s0:s0 + SPB * KPS].max(axis=1)
            for ridx, t in enumerate(R_T):
                rmax[b, t] = craw[:, b, ridx, :, :].astype(np.float32).max(axis=(1, 2))
        dist1_sq[:, c * GPC:(c + 1) * GPC] = -rmax.reshape(B, GPC).astype(np.float64)

        # col-side: max of chain-fold slots and raw spans over tiles
        raw_part = craw.max(axis=2)                       # [128, B, 2, SPANW]
        part = np.maximum(cfold, raw_part).max(axis=0)    # [B, 2, SPANW]
        dist2_parts.append(part)

    cmax = np.stack(dist2_parts, axis=0).max(axis=0)     # [B, 2, SPANW]
    cmax = cmax.reshape(B, N)                            # sp-major cols
    dist2_sq = -(cmax.astype(np.float64))

    dist1 = np.sqrt(np.maximum(dist1_sq, 0.0))
    dist2 = np.sqrt(np.maximum(dist2_sq, 0.0))
    chamfer = (dist1.mean(axis=1) + dist2.mean(axis=1)).mean()
    return np.float32(chamfer)


def kernel(pred, gt):
    _ensure_concourse()
    pred = np.asarray(pred, dtype=np.float32)
    gt = np.asarray(gt, dtype=np.float32)
    assert pred.shape == (B, N, 3) and gt.shape == (B, N, 3)

    in_maps = make_in_maps(pred, gt)
    nc = _get_nc()
    from concourse import bass_utils
    res = bass_utils.run_bass_kernel_spmd(nc, in_maps, core_ids=list(range(NCORES)))
    return finalize(res.results)


# revision 8
# speedup vs baseline: 1.1770x; 1.0696x over previous
"""Chamfer distance kernel for Trainium2 (8 NeuronCores).

Strategy (v5: 1024-wide strips / 4 PSUM slots to kill the 2-slot
latency chain; dense input DMA; memset-sourced PE warmup)
---------------------------------------------------------------------
dist[b,i,j] = ||pred[b,j] - gt[b,i]||.  Mins are taken over *negated
squared* distances (so every reduction is a max); sqrt and the means
happen on the host.

neg_sq[i,j] = 2*gt[i].pred[j] - |gt[i]|^2 - |pred[j]|^2 is produced
directly in PSUM by augmented K=24 bf16 matmuls (fp32 operands split
into bf16 triples; see _build_aug).  Operands are replicated at
partition bases 0/32/64/96; each 1024-col strip issues 2 concurrent
512-col matmuls in two of the four 32-row PE groups (groups alternate
with strip parity so adjacent strips overlap 4-wide in the PE).

v4 used [128,2048] strips = 4 PSUM banks, so only 2 slots fit and the
per-slot serial chain (evict -> MM latency+sems ~0.93us -> evict)
bounded the steady state at ~97us.  v5 uses [128,1024] strips = 2
banks -> 4 slots; the MM round trip hides behind the other 3 slots.
Cost model: ACT evict 1024x0.833+~160 = ~1.01us/strip, DVE fused
evict+rowmax 1024x1.042+~120 = ~1.18us/strip (PSUM fp32 reads are 1x;
bf16 PSUM would enable 2x DVE but is TRN3-only).

Sharding: gt rows split across 8 cores (1024 rows/core/batch = 8 row
tiles of 128).  Per batch, tiles are R-type (5: ACT copy-evict, fp16
span halves ship raw to DRAM, host reduces rows+cols) or F-type (3:
DVE fused evict+rowmax; spans chain-fold via fp16 tensor_tensor 2x and
the fold ships).  ACT 80 strips x 1.01 = 80.8us ~= DVE 48 x 1.18 +
8 folds x 2.29 + accum reads = 78.9us ~= DMA 28.9MB / 0.376MB/us =
76.7us -- three-way balanced.

Startup: input DMA ships only the 96 live partition rows (ag via 4
sync-queue dispatches, apt via 8 gpsimd/SWDGE dispatches split by
(group, batch)), and the PE warmup matmuls read a memset tile instead
of waiting for ag, so real strips start as soon as b0's columns land.
"""

import os
import sys
import numpy as np
import ml_dtypes

# ---------------------------------------------------------------------------
# problem constants (hardcoded per spec: pred/gt [2, 8192, 3] fp32)
B = 2
N = 8192
NCORES = 8
GPC = N // NCORES          # gt rows per core per batch = 1024
RT = GPC // 128            # row tiles per batch per core = 8
SPANW = 4096               # span width (ship/fold unit)
SPB = 2                    # spans per tile (8192 / 4096)
KPS = 4                    # strips per span (4096 / 1024)
W = 1024                   # strip width (2 PSUM banks -> 4 slots)
NSTRIP = B * RT * SPB * KPS  # 128 strips per core
NTILE = B * RT             # 16 row tiles per core
K = 24                     # contraction rows of the augmented matmul

# per-batch tile types: F (DVE fused evict+rowmax, col-side via fold
# chain) at {0,4,6}; R (ACT copy-evict + raw fp16 ship, host reduces)
# elsewhere.  Engine interleave below.
F_T = (0, 4, 6)
R_T = (1, 2, 3, 5, 7)
# Period-16 strip interleave (10 ACT : 6 DVE = 40:24 per 64-strip
# batch, exactly matching 5 R + 3 F tiles x 8 strips).  Chosen by
# discrete-event search over the 4-slot pipeline.
PAT16 = "AADDAADADADADAAA"
# final window of the stream: alternate D/A so the last fold + cmax ship
# happen ~7us before the last eviction and the trailing DMA is covered
PAT16_LAST = "DADADADADADAAAAA"
# PE HAM warmup matmuls before the main loop.  The burst must be long
# enough (~9us serial) to cross the clock-gate's un-throttle threshold;
# with a short burst the whole core runs ~17% slower for the entire
# execution (measured: MM 724 vs 604, ACT 1336 vs 1139).
WARMUP_MM = 22

_BF16 = ml_dtypes.bfloat16


def _ensure_concourse():
    for p in ("/root/.axon_site", "/root/.axon_site/_ro/trn_rl_repo",
              "/root/.axon_site/_ro/pypackages", "/opt/trn_rl_repo"):
        if os.path.isdir(p) and p not in sys.path:
            sys.path.append(p)


def _split3(x64):
    """Split a float64 array into three bf16 components summing to ~24 bits."""
    h = x64.astype(_BF16)
    r = x64 - h.astype(np.float64)
    m = r.astype(_BF16)
    r2 = r - m.astype(np.float64)
    l = r2.astype(_BF16)
    return h, m, l


def _build_aug(pred, gt):
    """Build aug_pred [K, B*N] and aug_gt [K, B*N] bf16 host arrays.

    Row pairing k: lhsT[k] (gt side) x rhs[k] (pred side):
      0-2   gh . Ph      3-5   gh . Pm      6-8   gm . Ph
      9-11  gh . Pl     12-14  gl . Ph     15-17  gm . Pm
      18-20 gsq{h,m,l} . (-1)              21-23  1 . (-psq{h,m,l})
    where P = 2*pred.
    """
    g64 = gt.astype(np.float64).reshape(B * N, 3)
    P64 = (2.0 * pred.astype(np.float64)).reshape(B * N, 3)
    gsq = (gt.astype(np.float32) ** 2).sum(-1, dtype=np.float32).astype(np.float64).reshape(B * N)
    psq = (pred.astype(np.float32) ** 2).sum(-1, dtype=np.float32).astype(np.float64).reshape(B * N)

    gh, gm, gl = _split3(g64)
    Ph, Pm, Pl = _split3(P64)
    gsqh, gsqm, gsql = _split3(gsq)
    psqh, psqm, psql = _split3(psq)

    one = np.ones(B * N, _BF16)
    neg1 = np.full(B * N, -1.0, _BF16)

    def rows3(a):  # [B*N, 3] -> 3 rows
        return [a[:, 0], a[:, 1], a[:, 2]]

    aug_gt = np.stack(
        rows3(gh) + rows3(gh) + rows3(gm) + rows3(gh) + rows3(gl) + rows3(gm)
        + [gsqh, gsqm, gsql, one, one, one], axis=0)
    aug_pred = np.stack(
        rows3(Ph) + rows3(Pm) + rows3(Ph) + rows3(Pl) + rows3(Ph) + rows3(Pm)
        + [neg1, neg1, neg1, -psqh, -psqm, -psql], axis=0)
    assert aug_gt.shape == (K, B * N) and aug_pred.shape == (K, B * N)
    return aug_gt, aug_pred


def build_nc():
    """Trace + compile the single-program SPMD kernel. Returns the Bacc."""
    _ensure_concourse()
    from contextlib import ExitStack
    import concourse.tile as tile
    from concourse import bacc, mybir

    f32 = mybir.dt.float32
    bf16 = mybir.dt.bfloat16
    f16 = mybir.dt.float16
    MAX = mybir.AluOpType.max
    ADD = mybir.AluOpType.add

    nc = bacc.Bacc("TRN2", target_bir_lowering=False, debug=False,
                   enable_asserts=False, num_devices=NCORES)
    # aug operands arrive pre-replicated at partition bases 0/32/64/96
    # (dead rows 24-31/56-63/88-95 exist in DRAM but are never DMA'd).
    ag_d = nc.dram_tensor("aug_gt", [96 + K, B * GPC], bf16,
                          kind="ExternalInput").ap()
    ap_d = nc.dram_tensor("aug_pred", [96 + K, B * N], bf16,
                          kind="ExternalInput").ap()
    # F tiles' fused rowmax accums, one column per strip id
    rmax_d = nc.dram_tensor("rowmax_out", [128, NSTRIP], f32, kind="ExternalOutput").ap()
    # chain-folded col-side spans: slot pid = b*2 + sp
    cmax_d = nc.dram_tensor("colmax_out", [128, B * 2 * SPANW], f16,
                            kind="ExternalOutput").ap()
    # raw spans of R tiles: slot = (b*5 + R_T.index(t))*2 + sp
    craw_d = nc.dram_tensor("colraw_out", [128, B * 5 * 2 * SPANW], f16,
                            kind="ExternalOutput").ap()

    with tile.TileContext(nc) as tc, ExitStack() as ctx:
        const_pool = ctx.enter_context(tc.tile_pool(name="const", bufs=1))
        psum_pool = ctx.enter_context(tc.tile_pool(name="ps", bufs=4, space="PSUM"))
        span_pool = ctx.enter_context(tc.tile_pool(name="bs", bufs=2))
        pf_pool = ctx.enter_context(tc.tile_pool(name="pf", bufs=3))

        ag = const_pool.tile([96 + K, B * GPC], bf16)
        apt = const_pool.tile([96 + K, B * N], bf16)
        # ag's first 1024 columns ship separately so the warmup (which only
        # reads them) starts ~1.5us before the rest of ag lands.
        nc.sync.dma_start(ag[:, 0:1024], ag_d[:, 0:1024])
        nc.sync.dma_start(ag[:, 1024:], ag_d[:, 1024:])
        for b in range(B):
            for cb in range(4):
                ccol = b * N + cb * 2048
                nc.sync.dma_start(apt[:, ccol:ccol + 2048],
                                  ap_d[:, ccol:ccol + 2048])
        rscr = const_pool.tile([128, NSTRIP], f32)
        nc.vector.memset(rscr[:], -3.0e38)

        # PE HAM warmup: sustained matmul activity (reading only ag's first
        # chunk) un-throttles the clock gate before the real strips start;
        # results are overwritten/ignored.  Rotating over 4 PE row groups x
        # 2 PSUM tiles x 2 halves makes the burst ~4-wide (dense ~5us
        # instead of a 9.4us serial chain).
        pw1 = psum_pool.tile([128, W], f32, tag="ps")
        pw2 = psum_pool.tile([128, W], f32, tag="ps")
        for n in range(WARMUP_MM):
            g = 32 * (n % 4)
            pw = pw1 if (n % 4) < 2 else pw2
            h = (n % 2) * 512
            nc.tensor.matmul(pw[:, h:h + 512], lhsT=ag[g:g + K, 0:128],
                             rhs=ag[g:g + K, 512:1024], start=True, stop=True,
                             tile_position=(g, 0))

        gstrip = [0]  # global strip counter (selects the PE group pair)

        def emit_strip(b, t, sp, k, span, typ):
            ti = b * RT + t
            s = (ti * SPB + sp) * KPS + k
            ccol = b * N + sp * SPANW + k * W
            wcol = ti * 128
            psum = psum_pool.tile([128, W], f32, tag="ps", name="psum")
            gbase = 64 * (gstrip[0] % 2)
            gstrip[0] += 1
            for j in range(2):
                g = gbase + 32 * j
                nc.tensor.matmul(
                    psum[:, j * 512:(j + 1) * 512],
                    lhsT=ag[g:g + K, wcol:wcol + 128],
                    rhs=apt[g:g + K, ccol + j * 512: ccol + (j + 1) * 512],
                    start=True, stop=True,
                    tile_position=(g, 0))
            out_slice = span[:, k * W:(k + 1) * W]
            if typ == 'F':
                # evict + this strip's rowmax in one 1x DVE pass
                nc.vector.tensor_scalar(
                    out=out_slice, in0=psum[:], scalar1=0.0,
                    scalar2=None, op0=ADD, op1=MAX,
                    accum_out=rscr[:, s:s + 1])
            else:
                nc.scalar.activation(out_slice, psum[:],
                                     mybir.ActivationFunctionType.Copy)

        def ship_half(b, t, sp, h, span):
            # raw R half-span ships on the GPSIMD (SWDGE) DMA queue so
            # they can't head-of-line block the sync queue's fold ships
            slot = ((b * 5 + R_T.index(t)) * 2 + sp) * SPANW
            nc.gpsimd.dma_start(
                craw_d[:, slot + h * 2048: slot + (h + 1) * 2048],
                span[:, h * 2048:(h + 1) * 2048])

        def finish_span_f(b, t, sp, span):
            # chain-fold the F spans of this (b, sp); ship after the third
            key = (b, sp)
            acc, cnt = chain.get(key, (None, 0))
            if acc is None:
                chain[key] = (span, 1)
                return
            pf = pf_pool.tile([128, SPANW], f16, tag="pf")
            nc.vector.tensor_tensor(out=pf[:], in0=acc[:], in1=span[:], op=MAX)
            cnt += 1
            if cnt < len(F_T):
                chain[key] = (pf, cnt)
            else:
                pid = b * 2 + sp
                nc.sync.dma_start(
                    cmax_d[:, pid * SPANW:(pid + 1) * SPANW], pf[:])
                chain.pop(key)

        # Flat per-batch strip stream: DVE-evicted (F) and ACT-evicted
        # (R) strips interleaved 10:6 per 16 so both engines run
        # continuously through the four PSUM slots.
        chain = {}  # (b, sp) -> (accumulated col-side fold tile, count)
        for b in range(B):
            # sp-major so each (b, sp) fold chain completes as early as its
            # last span allows (and early strips touch only sp0's columns)
            astrips = [(t, sp, k) for sp in range(SPB) for t in R_T
                       for k in range(KPS)]
            dstrips = [(t, sp, k) for sp in range(SPB) for t in F_T
                       for k in range(KPS)]
            ai = di = 0
            cur = {}       # (t, sp) -> span being filled
            for i in range(len(astrips) + len(dstrips)):
                last_window = (b == B - 1) and i >= 48
                which = (PAT16_LAST if last_window else PAT16)[i % 16]
                if (which == 'D' and di < len(dstrips)) or ai >= len(astrips):
                    t, sp, k = dstrips[di]
                    di += 1
                    side = 'd'
                else:
                    t, sp, k = astrips[ai]
                    ai += 1
                    side = 'a'
                typ = 'F' if t in F_T else 'R'
                if k == 0:
                    cur[(t, sp)] = span_pool.tile([128, SPANW], f16,
                                                  tag=f"sp_{side}",
                                                  bufs=6 if side == 'a' else 4,
                                                  name="span")
                emit_strip(b, t, sp, k, cur[(t, sp)], typ)
                if typ == 'R' and k in (1, 3):
                    ship_half(b, t, sp, k // 2, cur[(t, sp)])
                    if k == 3:
                        cur.pop((t, sp))
                elif typ == 'F' and k == 3:
                    finish_span_f(b, t, sp, cur.pop((t, sp)))
        nc.sync.dma_start(rmax_d[:], rscr[:])

    nc.compile()
    return nc


_NC_CACHE = None


def _get_nc():
    global _NC_CACHE
    if _NC_CACHE is None:
        _NC_CACHE = build_nc()
    return _NC_CACHE


def _replicate4(a):
    """Replicate [K, cols] to partition bases 0/32/64/96 of a [96+K, cols]."""
    out = np.zeros((96 + K, a.shape[1]), _BF16)
    for g in range(4):
        out[32 * g:32 * g + K] = a
    return out


def make_in_maps(pred, gt):
    """Per-core input dicts. Core c gets gt rows [c*GPC, (c+1)*GPC) of each
    batch (aug_gt columns laid out b-major: (b*RT + t)*128 + p)."""
    aug_gt, aug_pred = _build_aug(pred, gt)
    ag_bn = aug_gt.reshape(K, B, N)
    apr = _replicate4(aug_pred)
    in_maps = []
    for c in range(NCORES):
        ag_c = ag_bn[:, :, c * GPC:(c + 1) * GPC].reshape(K, B * GPC)
        in_maps.append({"aug_gt": _replicate4(ag_c), "aug_pred": apr})
    return in_maps


def finalize(results):
    """Host finale: negated maxes -> mins -> sqrt -> means."""
    dist1_sq = np.empty((B, N), np.float64)
    dist2_parts = []   # per-core [B, 2, SPANW] col-side partial maxes
    for c in range(NCORES):
        rscr = np.asarray(results[c]["rowmax_out"], np.float32)
        # colraw_out layout: [128, ((b*5+ridx)*2+sp)*SPANW + jj]
        craw = np.asarray(results[c]["colraw_out"]).reshape(128, B, 5, 2, SPANW)
        cfold = np.asarray(results[c]["colmax_out"]).reshape(128, B, 2, SPANW)

        rmax = np.empty((B, RT, 128), np.float32)
        for b in range(B):
            for t in F_T:
                s0 = (b * RT + t) * SPB * KPS
                rmax[b, t] = rscr[:, s0:s0 + SPB * KPS].max(axis=1)
            for ridx, t in enumerate(R_T):
                rmax[b, t] = craw[:, b, ridx, :, :].astype(np.float32).max(axis=(1, 2))
        dist1_sq[:, c * GPC:(c + 1) * GPC] = -rmax.reshape(B, GPC).astype(np.float64)

        # col-side: max of chain-fold slots and raw spans over tiles
        raw_part = craw.max(axis=2)                       # [128, B, 2, SPANW]
        part = np.maximum(cfold, raw_part).max(axis=0)    # [B, 2, SPANW]
        dist2_parts.append(part)

    cmax = np.stack(dist2_parts, axis=0).max(axis=0)     # [B, 2, SPANW]
    cmax = cmax.reshape(B, N)                            # sp-major cols
    dist2_sq = -(cmax.astype(np.float64))

    dist1 = np.sqrt(np.maximum(dist1_sq, 0.0))
    dist2 = np.sqrt(np.maximum(dist2_sq, 0.0))
    chamfer = (dist1.mean(axis=1) + dist2.mean(axis=1)).mean()
    return np.float32(chamfer)


def kernel(pred, gt):
    _ensure_concourse()
    pred = np.asarray(pred, dtype=np.float32)
    gt = np.asarray(gt, dtype=np.float32)
    assert pred.shape == (B, N, 3) and gt.shape == (B, N, 3)

    in_maps = make_in_maps(pred, gt)
    nc = _get_nc()
    from concourse import bass_utils
    res = bass_utils.run_bass_kernel_spmd(nc, in_maps, core_ids=list(range(NCORES)))
    return finalize(res.results)
